# revision 10
# baseline (speedup 1.0000x reference)
"""BitNet attention (D_MODEL=2048, 16 heads, B=2, T=2048) on 8 TRN2 cores. v3

Sharding: tensor-parallel over heads — each core owns 2 heads (256 dims) of
the q/k/v projections (column-parallel) and 256 output columns of out_proj
(column-parallel on a full, AllGather-ed int8-quantized activation).

Pipeline (emission order == per-engine queue order):
  P0a: weight absmeans + AllGather issue (AllReduce is pathologically slow)
  b0: P1 (act-quant x, SBUF->SBUF xbar transposes)
      P0b (weight quantize; gathered sums tree-summed here)
      P2 (K, V, Q projections)
      P3(c0) attn | P4a(c0) rmax+AGsm | P3(c1) | P4a(c1)
      P4b(c0) quant+AGbig | P4b(c1)
  b1: P1, P2, P3(c0), P4a(c0), P3(c1), P4a(c1),
      P4b(c0), P5(b0 c0+c1) [PE filler under b1's AllGathers], P4b(c1)
  P5(b1 c0+c1)

Numerics identical to v1/v2 (bit-exact): integer bf16 projections, f32r
(FP22) attention at full PE rate, magic-number RNE rounding, softmax without
max-subtraction, int8 exact quantized activations.
"""

import numpy as np

import concourse.bass as bass
import concourse.mybir as mybir
import concourse.tile as tile
from concourse.bass_utils import run_bass_kernel_spmd
from concourse.vector_clock import ScopedClock

DT = mybir.dt
ALU = mybir.AluOpType
ACTF = mybir.ActivationFunctionType

N_CORES = 8
P = 128
FD = 2048          # d_model
B, T = 2, 2048
BT = B * T
OC = FD // N_CORES  # 256
NF = FD // P        # 16 feature tiles
NT = T // P         # 16 token tiles per batch
TB = 512            # attention t1 block / P5 sub-chunk
NTB = T // TB       # 4
CH = 2              # P4/P5 chunks per batch
TC = T // CH        # 1024 (legacy; chunk sizes come from CHB)
NC = TC // P        # 8
CHB = ((0, 8), (8, 8))  # (tile offset, tile count) per chunk
MAGIC = 12582912.0  # 1.5 * 2**23
RG = [list(range(N_CORES))]

# --- walrus workarounds (same as v1) ---------------------------------------
_orig_drain_and_barrier = tile.TileContext._drain_and_barrier


def _patched_drain_and_barrier(self, tick_clock, wait_clock):
    nc = self.nc
    drain_inst = nc.sync.drain()
    wait_clock.add_sem_waits(
        drain_inst.ins, ScopedClock({None: tick_clock.global_clock})
    )
    si = drain_inst.ins.sync_info
    waits = list(si.on_wait or []) if si is not None else []
    if len(waits) > 1:
        si.on_wait = waits[:1]
        for w in waits[1:]:
            extra = nc.sync.drain()
            extra.ins.sync_info = mybir.SyncInfo(on_wait=[w], on_update=[])

    nc.all_engine_barrier()
    assert self.sems is not None
    popped = nc._tile_sem_poison_stack.pop()
    assert popped is self._sem_poison
    nc.clear_and_free_semaphores(list(self.sems.allocated().values()))
    nc.all_engine_barrier()


def _install_patch():
    tile.TileContext._drain_and_barrier = _patched_drain_and_barrier


def _split_excess_waits(nc, max_waits):
    n_new = 0
    for fn in nc.m.functions:
        for blk in fn.blocks:
            il = blk.instructions
            out = []
            for inst in il:
                si = getattr(inst, "sync_info", None)
                waits = list(si.on_wait) if (si is not None and si.on_wait) else []
                if len(waits) > max_waits:
                    extra = waits[:-max_waits] if max_waits else waits
                    keep = waits[-max_waits:] if max_waits else []
                    step = max(1, max_waits)
                    for k in range(0, len(extra), step):
                        n_new += 1
                        nop = mybir.InstNoOp(
                            name=f"WSP{n_new}",
                            sync_info=mybir.SyncInfo(
                                on_wait=extra[k:k + step], on_update=[]),
                            bass_nofuse=True,
                            engine=inst.engine,
                        )
                        nc.register_instruction(nop, overwrite=True)
                        out.append(nop)
                    si.on_wait = keep
                out.append(inst)
            il[:] = out
    return n_new


# ---------------------------------------------------------------------------
fp32 = DT.float32
bf16 = DT.bfloat16
f32r = DT.float32r
i8 = DT.int8
X = mybir.AxisListType.X


def build_kernel():
    import os
    _install_patch()
    nc = bass.Bass("TRN2", target_bir_lowering=False, debug=False,
                   num_devices=N_CORES)
    x_in = nc.dram_tensor("x", [BT, FD], DT.float32, kind="ExternalInput")
    wT = {
        w: nc.dram_tensor(f"w{w}T", [FD, OC], DT.float32, kind="ExternalInput")
        for w in "qkvo"
    }
    y_out = nc.dram_tensor("y", [B, T, OC], DT.float32, kind="ExternalOutput")

    with tile.TileContext(nc) as tc:
        for _rep in range(int(os.environ.get("KREPEAT", "1"))):
            _top(nc, tc, x_in, wT, y_out)
    _split_excess_waits(nc, int(os.environ.get("BASS_MAX_WAITS", "1")))
    return nc


class Ctx:
    pass


def _top(nc, tc, x_in, wT, y_out):
    from contextlib import ExitStack
    with ExitStack() as stack:
        g = Ctx()
        g.nc, g.tc, g.x_in, g.wT, g.y_out = nc, tc, x_in, wT, y_out
        g.const = stack.enter_context(tc.tile_pool(name="const", bufs=1))
        g.dram = stack.enter_context(tc.tile_pool(name="dram", bufs=1, space="DRAM"))
        g.persist = stack.enter_context(tc.tile_pool(name="persist", bufs=1))

        const = g.const
        g.ones_col = const.tile([P, 1], fp32, tag="ones_col", name="ones_col")
        nc.gpsimd.memset(g.ones_col[:], 1.0)
        g.ones_row = const.tile([1, P], fp32, tag="ones_row", name="ones_row")
        nc.gpsimd.memset(g.ones_row[:], 1.0)
        g.ones_col_r = const.tile([P, 1], f32r, tag="ones_col_r", name="ones_col_r")
        nc.vector.tensor_copy(g.ones_col_r[:], g.ones_col[:])
        g.ones_row_r = const.tile([1, P], f32r, tag="ones_row_r",
                                  name="ones_row_r")
        nc.vector.tensor_copy(g.ones_row_r[:], g.ones_row[:])
        g.ident = const.tile([P, P], fp32, tag="ident", name="ident")
        from concourse.masks import make_identity
        make_identity(nc, g.ident[:])

        g.wqbf = {
            w: [const.tile([P, OC], bf16, tag=f"w{w}bf{i}", name=f"w{w}bf{i}")
                for i in range(NF)]
            for w in "qkvo"
        }
        g.swb = const.tile([P, 8], fp32, tag="swb", name="swb")
        g.magicv = const.tile([P, 1], fp32, tag="magicv", name="magicv")
        nc.gpsimd.memset(g.magicv[:], MAGIC)
        g.nmagicv = const.tile([P, 1], fp32, tag="nmagicv", name="nmagicv")
        nc.gpsimd.memset(g.nmagicv[:], -MAGIC)
        g.cvec = const.tile([P, 1], fp32, tag="cvec", name="cvec")

        _p0a(g)
        bst = {}
        for b in range(B):
            bst[b] = Ctx()
            _batch(g, b, bst[b],
                   between=(lambda: _p0b(g)) if b == 0 else None,
                   filler=(lambda bb=b: _p5(g, bst[bb - 1])) if b > 0 else None)
        _p5(g, bst[B - 1])


def _p0a(g):
    """Weight absmeans + AllGather issue. Result consumed in _p0b."""
    nc, tc = g.nc, g.tc
    g.cc_out = g.dram.tile([N_CORES, 4], fp32, tag="cc_out", name="cc_out",
                           addr_space="Shared")
    cc_in = g.dram.tile([1, 4], fp32, tag="cc_in", name="cc_in")
    with (
        tc.tile_pool(name="wstage0", bufs=3) as wstage,
        tc.tile_pool(name="p0a", bufs=1) as p0,
        tc.tile_pool(name="p0aps", bufs=1, space="PSUM") as p0ps,
    ):
        asum = p0.tile([P, 4 * NF], fp32, tag="asum", name="asum")
        tots = p0.tile([P, 4], fp32, tag="tots", name="tots")
        for wi, w in [(1, "k"), (0, "q"), (2, "v"), (3, "o")]:
            for i4 in range(NF // 4):
                st = wstage.tile([P, 4 * OC], fp32, tag="wst", name="wst")
                eng = nc.scalar if (i4 % 2) else nc.sync
                eng.dma_start(
                    out=st[:].rearrange("p (a n) -> p a n", a=4),
                    in_=g.wT[w][i4 * 4 * P:(i4 + 1) * 4 * P, :].rearrange(
                        "(a p) n -> p a n", p=P))
                for k in range(4):
                    i = i4 * 4 + k
                    nc.vector.tensor_reduce(
                        asum[:, wi * NF + i: wi * NF + i + 1],
                        st[:, k * OC:(k + 1) * OC],
                        X, ALU.add, apply_absolute_value=True,
                    )
            nc.vector.tensor_reduce(
                tots[:, wi:wi + 1], asum[:, wi * NF:(wi + 1) * NF], X, ALU.add)
        ps14 = p0ps.tile([1, 4], fp32, tag="ps14", name="ps14")
        nc.tensor.matmul(ps14[:], lhsT=g.ones_col[:], rhs=tots[:],
                         start=True, stop=True)
        sums4 = p0.tile([1, 4], fp32, tag="sums4", name="sums4")
        nc.vector.tensor_copy(sums4[:], ps14[:])
        nc.sync.dma_start(out=cc_in[:], in_=sums4[:])
        nc.gpsimd.collective_compute(
            "AllGather", ALU.bypass, replica_groups=RG,
            ins=[cc_in.opt()], outs=[g.cc_out.opt()],
        )


def _p0b(g):
    """Consume weight-sum AllGather (tree-sum); build scales; quantize weights."""
    nc, tc = g.nc, g.tc
    with (
        tc.tile_pool(name="wstage1", bufs=2) as wstage,
        tc.tile_pool(name="p0b", bufs=1) as p0,
        tc.tile_pool(name="p0bps", bufs=1, space="PSUM") as p0ps,
    ):
        row8 = p0.tile([1, 8], fp32, tag="row8", name="row8")
        g32 = p0.tile([1, N_CORES * 4], fp32, tag="g32", name="g32")
        nc.gpsimd.dma_start(
            out=g32[:], in_=g.cc_out.rearrange("c n -> (c n)").unsqueeze(0))
        s16 = p0.tile([1, 16], fp32, tag="s16", name="s16")
        nc.vector.tensor_tensor(s16[:], g32[:, 0:16], g32[:, 16:32], ALU.add)
        s8 = p0.tile([1, 8], fp32, tag="s8", name="s8")
        nc.vector.tensor_tensor(s8[:], s16[:, 0:8], s16[:, 8:16], ALU.add)
        gs = p0.tile([1, 4], fp32, tag="gs", name="gs")
        nc.vector.tensor_tensor(gs[:], s8[:, 0:4], s8[:, 4:8], ALU.add)
        nc.vector.tensor_scalar(row8[:, 4:8], gs[:], 1.0 / (FD * FD), 1e-5,
                                ALU.mult, ALU.max)
        nc.vector.reciprocal(row8[:, 0:4], row8[:, 4:8])
        psb = p0ps.tile([P, 8], fp32, tag="psb", name="psb")
        nc.tensor.matmul(psb[:], lhsT=g.ones_row[:], rhs=row8[:],
                         start=True, stop=True)
        nc.vector.tensor_copy(g.swb[:], psb[:])
        cv1 = p0.tile([P, 1], fp32, tag="cv1", name="cv1")
        nc.vector.tensor_tensor(cv1[:], g.swb[:, 4:5], g.swb[:, 5:6], ALU.mult)
        nc.vector.tensor_scalar_mul(g.cvec[:], cv1[:], float(P) ** -0.5)

        for wi, w in [(1, "k"), (0, "q"), (2, "v"), (3, "o")]:
            for i4 in range(NF // 4):
                st = wstage.tile([P, 4 * OC], fp32, tag="wst", name="wst")
                nc.scalar.dma_start(
                    out=st[:].rearrange("p (a n) -> p a n", a=4),
                    in_=g.wT[w][i4 * 4 * P:(i4 + 1) * 4 * P, :].rearrange(
                        "(a p) n -> p a n", p=P))
                for k in range(4):
                    i = i4 * 4 + k
                    ss = st[:, k * OC:(k + 1) * OC]
                    t1 = wstage.tile([P, OC], fp32, tag="wt1", name="wt1")
                    nc.vector.tensor_scalar(t1[:], ss, g.swb[:, wi:wi + 1],
                                            MAGIC, ALU.mult, ALU.add)
                    t2 = wstage.tile([P, OC], fp32, tag="wt2", name="wt2")
                    nc.vector.tensor_scalar(t2[:], t1[:], -MAGIC, 1.0,
                                            ALU.add, ALU.min)
                    nc.vector.tensor_scalar_max(g.wqbf[w][i][:], t2[:], -1.0)


def _batch(g, b, bst, between=None, filler=None):
    nc, tc = g.nc, g.tc
    from contextlib import ExitStack
    with ExitStack() as bstack:
        pA = bstack.enter_context(tc.tile_pool(name=f"A{b}", bufs=1))
        QT = [pA.tile([P, T], f32r, tag=f"QT{h}", name=f"QT{h}_{b}") for h in range(2)]
        KT = [pA.tile([P, T], f32r, tag=f"KT{h}", name=f"KT{h}_{b}") for h in range(2)]
        V = [pA.tile([P, OC], f32r, tag=f"V{j}", name=f"V{j}_{b}") for j in range(NT)]

        # ---------------- P1 (+ optional `between`) + P2 ----------------
        with tc.tile_pool(name=f"A2_{b}", bufs=1) as pA2:
            sinv = pA2.tile([P, NT], fp32, tag="sinv", name=f"sinv{b}")
            R = pA2.tile([P, T], fp32, tag="R", name=f"R{b}")
            qxTb = pA2.tile([P, NF * T], bf16, tag="qxTb", name=f"qxTb{b}")
            qxT3 = qxTb[:].rearrange("p (i t) -> p i t", i=NF)

            def qxs(i, lo, hi):
                return qxTb[:, i * T + lo: i * T + hi]

            with (
                tc.tile_pool(name=f"xstage{b}", bufs=4) as xstage,
                tc.tile_pool(name=f"p1s{b}", bufs=4) as p1s,
                tc.tile_pool(name=f"qb{b}", bufs=2) as qbp,
            ):
              srows = [p1s.tile([1, TB], fp32, tag=f"srow{t4}",
                                name=f"srow{b}_{t4}", bufs=1)
                       for t4 in range(NTB)]
              with (
                  tc.tile_pool(name=f"tqps{b}", bufs=3, space="PSUM") as tqps,
                  tc.tile_pool(name=f"rps{b}", bufs=1, space="PSUM") as rps,
              ):
                  PF = 3  # x-load prefetch depth
                  xts = {}

                  def ldx(j):
                      if j >= NT:
                          return
                      xt = xstage.tile([P, FD], fp32, tag="xt", name="xt")
                      nc.sync.dma_start(
                          out=xt[:],
                          in_=g.x_in[b * T + j * P: b * T + (j + 1) * P, :])
                      xts[j] = xt

                  for j in range(PF):
                      ldx(j)
                  back = [None]
                  for j in range(NT):
                      xt = xts.pop(j)
                      am = p1s.tile([P, 1], fp32, tag="am", name="am")
                      nc.vector.tensor_reduce(am[:], xt[:], X, ALU.max,
                                              apply_absolute_value=True)
                      amc = p1s.tile([P, 1], fp32, tag="amc", name="amc")
                      nc.vector.tensor_scalar_max(amc[:], am[:], 1e-5)
                      sv = p1s.tile([P, 1], fp32, tag="sv", name="sv")
                      nc.vector.reciprocal(sv[:], amc[:])
                      svec = p1s.tile([P, 1], fp32, tag="svec", name="svec")
                      nc.vector.tensor_scalar_mul(svec[:], sv[:], 127.0)
                      nc.vector.tensor_scalar_mul(sinv[:, j:j + 1], amc[:],
                                                  1.0 / 127.0)
                      qb = qbp.tile([P, FD], bf16, tag="qb", name="qb")
                      tq0 = tqps.tile([P, FD // 2], fp32, tag="tq", name="tq")
                      nc.scalar.activation(tq0[:], xt[:, 0:FD // 2],
                                           ACTF.Identity, scale=svec[:],
                                           bias=g.magicv[:])
                      ldx(j + PF)
                      # back-half of the previous tile: emitted after this
                      # tile's front so the DVE/Act queues pipeline across tiles
                      if back[0] is not None:
                          back[0]()
                      tq1 = tqps.tile([P, FD // 2], fp32, tag="tq", name="tq")
                      nc.scalar.activation(tq1[:], xt[:, FD // 2:],
                                           ACTF.Identity, scale=svec[:],
                                           bias=g.magicv[:])
                      def _back(j=j, qb=qb, tqs=(tq0, tq1)):
                          nc.vector.tensor_scalar_add(
                              qb[:, 0:FD // 2], tqs[0][:], -MAGIC)
                          nc.scalar.activation(qb[:, FD // 2:], tqs[1][:],
                                               ACTF.Identity, bias=g.nmagicv[:])
                          nc.sync.dma_start(
                              out=qxT3[:, :, j * P:(j + 1) * P],
                              in_=qb[:],
                              transpose=True,
                          )
                      back[0] = _back
                      if j % 4 == 3:
                          # incremental R build for t4 = j//4: transpose the
                          # 4 fresh sinv columns to a row, broadcast via ones
                          t4 = j // 4
                          seg = slice(t4 * TB, (t4 + 1) * TB)
                          pstR = rps.tile([4, P], fp32, tag="pstR", name="pstR")
                          nc.tensor.transpose(
                              pstR[:], sinv[:, t4 * 4:(t4 + 1) * 4], g.ident[:])
                          sseg = p1s.tile([4, P], fp32, tag="sseg", name="sseg")
                          nc.vector.tensor_copy(sseg[:], pstR[:])
                          srow = srows[t4]
                          nc.sync.dma_start(out=srow[:], in_=sseg[:])
                          psR = rps.tile([P, TB], fp32, tag="Rmm", name="psR")
                          nc.tensor.matmul(psR[:], lhsT=g.ones_row_r[:],
                                           rhs=srow[:].bitcast(f32r),
                                           start=True, stop=True)
                          nc.vector.tensor_copy(R[:, seg], psR[:])
                  back[0]()


            if between is not None:
                between()

            # ---------------- P2 ----------------
            with (
                tc.tile_pool(name=f"qkps{b}", bufs=3, space="PSUM") as qkps,
                tc.tile_pool(name=f"vps{b}", bufs=2, space="PSUM") as vps,
            ):
                for wname, dst in (("k", KT), ("q", QT)):
                    for o in range(2):
                        for t4 in range(NTB):
                            ps = qkps.tile([P, TB], fp32, tag="qk", name="qk")
                            for i in range(NF):
                                nc.tensor.matmul(
                                    ps[:],
                                    lhsT=g.wqbf[wname][i][:, o * P:(o + 1) * P],
                                    rhs=qxs(i, t4 * TB, (t4 + 1) * TB),
                                    start=(i == 0), stop=(i == NF - 1),
                                )
                            nc.vector.tensor_tensor(
                                dst[o][:, t4 * TB:(t4 + 1) * TB], ps[:],
                                R[:, t4 * TB:(t4 + 1) * TB], ALU.mult,
                            )
                for j in range(NT):
                    ps = vps.tile([P, OC], fp32, tag="v", name="v")
                    for i in range(NF):
                        nc.tensor.matmul(
                            ps[:],
                            lhsT=qxs(i, j * P, (j + 1) * P),
                            rhs=g.wqbf["v"][i][:],
                            start=(i == 0), stop=(i == NF - 1),
                        )
                    nc.vector.tensor_scalar(V[j][:], ps[:], sinv[:, j:j + 1],
                                            g.swb[:, 6:7], ALU.mult, ALU.mult)

        # ---------------- P3/P4 state ----------------
        pB = bstack.enter_context(tc.tile_pool(name=f"B{b}", bufs=1))
        PVT = [pB.tile([P, T], fp32, tag=f"PVT{h}", name=f"PVT{h}_{b}")
               for h in range(2)]
        den = [pB.tile([1, T], fp32, tag=f"den{h}", name=f"den{h}_{b}")
               for h in range(2)]
        p4 = bstack.enter_context(tc.tile_pool(name=f"p4_{b}", bufs=1))
        bst.Ry = g.persist.tile([P, NT], fp32, tag=f"Ry{b}", name=f"Ry_{b}")
        bst.gath = []
        bst.b_idx = b
        st = Ctx()
        st.PVT, st.den, st.p4, st.b = PVT, den, p4, b

        for c in range(CH):
            # ---------------- P3 chunk ----------------
            with (
                tc.tile_pool(name=f"sps{b}_{c}", bufs=3, space="PSUM") as sps,
                tc.tile_pool(name=f"pvps{b}_{c}", bufs=2, space="PSUM") as pvps,
                tc.tile_pool(name=f"dnps{b}_{c}", bufs=2, space="PSUM") as dnps,
                tc.tile_pool(name=f"expp{b}_{c}", bufs=14) as expp,
            ):
                for hl in range(2):
                    for t1b in range(CHB[c][0] // 4,
                                     (CHB[c][0] + CHB[c][1]) // 4):
                        t1s = slice(t1b * TB, (t1b + 1) * TB)
                        pv = pvps.tile([P, TB], fp32, tag="pv", name="pv")
                        dn = dnps.tile([1, TB], fp32, tag="dn", name="dn")
                        for j in range(NT):
                            ss = sps.tile([P, TB], fp32, tag="ss", name="ss")
                            nc.tensor.matmul(
                                ss[:],
                                lhsT=KT[hl][:, j * P:(j + 1) * P],
                                rhs=QT[hl][:, t1s],
                                start=True, stop=True,
                            )
                            ex = expp.tile([P, TB], f32r, tag="ex", name="ex")
                            nc.scalar.activation(ex[:], ss[:], ACTF.Exp,
                                                 scale=g.cvec[:])
                            nc.tensor.matmul(
                                pv[:],
                                lhsT=V[j][:, hl * P:(hl + 1) * P],
                                rhs=ex[:],
                                start=(j == 0), stop=(j == NT - 1),
                            )
                            nc.tensor.matmul(
                                dn[:],
                                lhsT=g.ones_col_r[:],
                                rhs=ex[:],
                                start=(j == 0), stop=(j == NT - 1),
                            )
                        nc.vector.tensor_copy(PVT[hl][:, t1s], pv[:])
                        nc.vector.tensor_copy(den[hl][:, t1s], dn[:])
            _p4a(g, st, c, bst)

        _p4b(g, st, 0, bst)
        _p4b(g, st, CH - 1, bst)
        if filler is not None:
            filler()


def _p4a(g, st, c, bst):
    """Pre-collective P4: denT (1/den, token-partition), raw rmax, AGsm."""
    nc, tc = g.nc, g.tc
    p4, PVT, den, b = st.p4, st.PVT, st.den, st.b
    t0k, ntk = CHB[c]
    TCc = ntk * P
    cs = slice(t0k * P, t0k * P + TCc)

    with (
        tc.tile_pool(name=f"p4a_{b}_{c}", bufs=2) as scr,
        tc.tile_pool(name=f"p4aps_{b}_{c}", bufs=2, space="PSUM") as tps,
    ):
        denT = []
        for hl in range(2):
            d_den = g.dram.tile([1, TCc], fp32, tag=f"d_den{b}{c}{hl}",
                                name=f"d_den{b}{c}{hl}")
            nc.sync.dma_start(out=d_den[:], in_=den[hl][:, cs])
            den16 = scr.tile([ntk, P], fp32, tag="den16", name="den16")
            nc.sync.dma_start(
                out=den16[:],
                in_=d_den.rearrange("o (j p) -> (o j) p", p=P))
            pstd = tps.tile([P, ntk], fp32, tag="dT", name="pstd", bufs=1)
            nc.tensor.transpose(pstd[:], den16[:], g.ident[0:ntk, 0:ntk])
            dT = p4.tile([P, ntk], fp32, tag=f"denT{c}{hl}",
                         name=f"denT{b}{c}{hl}")
            nc.vector.reciprocal(dT[:], pstd[:])
            denT.append(dT)

        rrm = [p4.tile([P, ntk], fp32, tag=f"rrm{c}{hl}", name=f"rrm{b}{c}{hl}")
               for hl in range(2)]
        for jj in range(ntk):
            j = t0k + jj
            for hl in range(2):
                pst = tps.tile([P, P], fp32, tag="t", name="t")
                nc.tensor.transpose(pst[:], PVT[hl][:, j * P:(j + 1) * P],
                                    g.ident[:])
                nc.vector.tensor_reduce(
                    rrm[hl][:, jj:jj + 1], pst[:], X,
                    ALU.max, apply_absolute_value=True)

        rm = p4.tile([P, ntk], fp32, tag=f"rm{c}", name=f"rm{b}{c}")
        m0 = scr.tile([P, ntk], fp32, tag="m0", name="m0")
        nc.vector.tensor_tensor(m0[:], rrm[0][:], denT[0][:], ALU.mult)
        nc.vector.tensor_tensor(rm[:], rrm[1][:], denT[1][:], ALU.mult)
        nc.vector.tensor_tensor(rm[:], rm[:], m0[:], ALU.max)

        d_rm_in = g.dram.tile([P, ntk], fp32, tag=f"d_rm_in{b}{c}",
                              name=f"d_rm_in{b}{c}")
        d_rm_out = g.dram.tile([N_CORES * P, ntk], fp32, tag=f"d_rm_out{b}{c}",
                               name=f"d_rm_out{b}{c}", addr_space="Shared")
        nc.sync.dma_start(out=d_rm_in[:], in_=rm[:])
        nc.gpsimd.collective_compute(
            "AllGather", ALU.bypass, replica_groups=RG,
            ins=[d_rm_in.opt()], outs=[d_rm_out.opt()],
        )
        st.__dict__[f"denT{c}"] = denT
        st.__dict__[f"d_rm_out{c}"] = d_rm_out


def _p4b(g, st, c, bst):
    """Post-collective P4: global rmax, scales, int8 quantize, AGbig."""
    nc, tc = g.nc, g.tc
    p4, PVT, b = st.p4, st.PVT, st.b
    denT = st.__dict__[f"denT{c}"]
    d_rm_out = st.__dict__[f"d_rm_out{c}"]
    t0k, ntk = CHB[c]
    TCc = ntk * P
    cs = slice(t0k * P, t0k * P + TCc)

    with (
        tc.tile_pool(name=f"p4b_{b}_{c}", bufs=2) as scr,
        tc.tile_pool(name=f"p4bps_{b}_{c}", bufs=1, space="PSUM") as bps,
    ):
        gath = scr.tile([P, N_CORES * ntk], fp32, tag="gath", name="gath")
        nc.sync.dma_start(
            out=gath[:].rearrange("p (c n) -> p c n", c=N_CORES),
            in_=d_rm_out.rearrange("(c p) n -> p c n", p=P))
        m4 = scr.tile([P, 4 * ntk], fp32, tag="m4", name="m4")
        nc.vector.tensor_tensor(m4[:], gath[:, 0:4 * ntk],
                                gath[:, 4 * ntk:8 * ntk], ALU.max)
        m2 = scr.tile([P, 2 * ntk], fp32, tag="m2", name="m2")
        nc.vector.tensor_tensor(m2[:], m4[:, 0:2 * ntk], m4[:, 2 * ntk:4 * ntk],
                                ALU.max)
        rmg = scr.tile([P, ntk], fp32, tag="rmg", name="rmg")
        nc.vector.tensor_tensor(rmg[:], m2[:, 0:ntk], m2[:, ntk:2 * ntk], ALU.max)
        mxt = scr.tile([P, ntk], fp32, tag="mxt", name="mxt")
        nc.vector.tensor_scalar_max(mxt[:], rmg[:], 1e-5)
        rc = scr.tile([P, ntk], fp32, tag="rc", name="rc")
        nc.vector.reciprocal(rc[:], mxt[:])
        sqa = scr.tile([P, ntk], fp32, tag="sqa", name="sqa")
        nc.vector.tensor_scalar_mul(sqa[:], rc[:], 127.0)
        nc.vector.tensor_scalar(bst.Ry[:, t0k:t0k + ntk], mxt[:],
                                g.swb[:, 7:8], 1.0 / 127.0, ALU.mult, ALU.mult)

        d_qaT8 = g.dram.tile([OC, TCc], i8, tag=f"d_qaT8_{b}{c}",
                             name=f"d_qaT8_{b}{c}")
        for hl in range(2):
            qs = scr.tile([P, ntk], fp32, tag=f"qs{hl}", name=f"qs{hl}")
            nc.vector.tensor_tensor(qs[:], sqa[:], denT[hl][:], ALU.mult)
            psq = bps.tile([ntk, P], fp32, tag="qsT", name="psq")
            nc.tensor.transpose(psq[:], qs[:], g.ident[:])
            qsT = scr.tile([ntk, P], fp32, tag=f"qsTs{hl}", name=f"qsT{hl}")
            nc.vector.tensor_copy(qsT[:], psq[:])
            qsrow = scr.tile([1, TCc], fp32, tag=f"qsrow{hl}", name=f"qsrow{hl}")
            nc.sync.dma_start(out=qsrow[:], in_=qsT[:])
            Sp = bps.tile([P, TCc], fp32, tag="S", name="Sp")
            for sb in range(TCc // TB):
                nc.tensor.matmul(Sp[:, sb * TB:(sb + 1) * TB],
                                 lhsT=g.ones_row_r[:],
                                 rhs=qsrow[:, sb * TB:(sb + 1) * TB].bitcast(
                                     f32r),
                                 start=True, stop=True)
            t1 = scr.tile([P, TCc], fp32, tag="qt1", name="qt1")
            nc.vector.tensor_tensor(t1[:], PVT[hl][:, cs], Sp[:], ALU.mult)
            t2 = scr.tile([P, TCc], fp32, tag="qt2", name="qt2")
            nc.vector.tensor_scalar_add(t2[:], t1[:], MAGIC)
            qa8 = scr.tile([P, TCc], i8, tag="qa8", name="qa8")
            nc.vector.tensor_scalar_add(qa8[:], t2[:], -MAGIC)
            nc.sync.dma_start(out=d_qaT8[hl * P:(hl + 1) * P, :], in_=qa8[:])

        d_qaTg = g.dram.tile([FD, TCc], i8, tag=f"d_qaTg{b}{c}",
                             name=f"d_qaTg{b}{c}", addr_space="Shared")
        nc.gpsimd.collective_compute(
            "AllGather", ALU.bypass, replica_groups=RG,
            ins=[d_qaT8.opt()], outs=[d_qaTg.opt()],
        )
        bst.gath.append(d_qaTg)


def _p5(g, bst):
    """out_proj, token-major; gathered int8 widened to bf16 by casting DMA."""
    nc, tc = g.nc, g.tc
    b = bst.b_idx
    with (
        tc.tile_pool(name=f"gbf_{b}", bufs=2) as gbfp,
        tc.tile_pool(name=f"yn_{b}", bufs=3) as ynp,
        tc.tile_pool(name=f"ops_{b}", bufs=4, space="PSUM") as ops,
    ):
        for c in range(CH):
            d_qaTg = bst.gath[c]
            t0k, ntk = CHB[c]
            for sub in range(ntk * P // TB):
                g8big = gbfp.tile([P, NF * TB], i8, tag="g8big", name="g8big")
                nc.sync.dma_start(
                    out=g8big[:].rearrange("p (i t) -> p i t", i=NF),
                    in_=d_qaTg[:, sub * TB:(sub + 1) * TB].rearrange(
                        "(i p) t -> p i t", p=P))
                gbig = gbfp.tile([P, NF * TB], bf16, tag="gbig", name="gbig")
                nc.vector.tensor_copy(gbig[:], g8big[:])
                for jj in range(TB // P):
                    j = t0k + sub * (TB // P) + jj
                    ps = ops.tile([P, OC], fp32, tag="o", name="o")
                    for i in range(NF):
                        nc.tensor.matmul(
                            ps[:],
                            lhsT=gbig[:, i * TB + jj * P: i * TB + (jj + 1) * P],
                            rhs=g.wqbf["o"][i][:],
                            start=(i == 0), stop=(i == NF - 1),
                        )
                    yn = ynp.tile([P, OC], fp32, tag="yn", name="yn")
                    nc.vector.tensor_scalar(yn[:], ps[:], bst.Ry[:, j:j + 1],
                                            None, ALU.mult)
                    nc.sync.dma_start(
                        out=g.y_out[b, j * P:(j + 1) * P, :], in_=yn[:])


# ---------------------------------------------------------------------------
_CACHE = {}


def _get_nc():
    if "nc" not in _CACHE:
        _CACHE["nc"] = build_kernel()
    return _CACHE["nc"]


def prepare_in_maps(x, w_q, w_k, w_v, w_o):
    xf = np.ascontiguousarray(np.asarray(x, np.float32).reshape(BT, FD))
    ws = {"q": w_q, "k": w_k, "v": w_v, "o": w_o}
    in_maps = []
    for c in range(N_CORES):
        m = {"x": xf}
        for k, w in ws.items():
            sl = np.asarray(w, np.float32)[c * OC:(c + 1) * OC, :]
            m[f"w{k}T"] = np.ascontiguousarray(sl.T)
        in_maps.append(m)
    return in_maps


def kernel(x, w_q, w_k, w_v, w_o):
    nc = _get_nc()
    in_maps = prepare_in_maps(x, w_q, w_k, w_v, w_o)
    last_err = None
    for _attempt in range(4):
        try:
            res = run_bass_kernel_spmd(nc, in_maps, list(range(N_CORES)))
            break
        except Exception as e:  # sporadic device-unrecoverable; retry
            last_err = e
            import time as _time
            _time.sleep(2.0)
    else:
        raise last_err
    outs = [res.results[c]["y"] for c in range(N_CORES)]  # [B, T, OC] each
    y = np.concatenate(outs, axis=2)  # [B, T, FD]
    return np.ascontiguousarray(y.astype(np.float32))

